# revision 1
# baseline (speedup 1.0000x reference)
"""Trainium2 Bass kernel for the SNN Leaky-Integrate-Fire problem.

Pipeline (per core, pure data-parallel over batch):
  cur1 = x @ W1.T + b1                        [B,32]  (PE fp32 matmul, bit-exact
                                                       vs the XLA-neuron reference)
  100x: mem = beta*mem + cur1 - H(mem-1)      (elementwise scan)
  spk  = H(mem - 1)
  out  = spk @ W2.T + b2                      [B,3]   (segmented reduce over h)

Numerics: the scan tracks n = -mem so each step is two fused
scalar_tensor_tensor ops (DVE lane) whose per-stage fp32 roundings match
the reference's  fl(fl(fl(beta*m)+cur1)-h)  sequence bit-for-bit:
  A  = (n * -beta) - cn         # cn = -cur1; A = fl(fl(beta*m)+cur1)
  n' = (n is_lt -1) - A         # n' = fl(h - A) = -m'
A second, independent column range runs on GPSIMD with the same values
via 4 plain tensor_tensor ops (STT is not in the Pool ISA and Pool
TENSOR_SCALAR is pathologically slow; TT with broadcast-constant views
is fast). Lanes use disjoint tiles so the engines never synchronize.

cur1 is computed on the PE with the exact operand layout the XLA-neuron
compiler uses for this matmul (stationary = x-chunk.T [3,128], moving =
W1.T [3,32], fp32 LOW/HIGH mode) -> bit-identical spikes.

Layout per core: 8192 rows; logical row r = chunk*128 + p lives at
partition p, free block chunk. Host feeds x_shard.T [3, 8192] and
inverse-permutes the output rows.
"""
import os
import sys

sys.path.insert(0, "/opt/trn_rl_repo")

import numpy as np

import concourse.bacc as bacc
import concourse.tile as tile
from concourse import mybir
from concourse.bass_utils import run_bass_kernel_spmd

F32 = mybir.dt.float32
ALU = mybir.AluOpType
AF = mybir.ActivationFunctionType

# problem constants (hardcoded per contract)
B, N_IN, N_HID, N_OUT = 65536, 3, 32, 3
NUM_STEPS, BETA, THR = 100, 0.9, 1.0
N_CORES = 8
BC = B // N_CORES          # rows per core = 8192
P = 128                    # partitions
NCH = BC // P              # 128-row chunks per core = 64
FREE = NCH * N_HID         # scan free size = 2048

# const block layout (replicated across partitions):
# [b1(32) w2(3*32) b2(3) pad(29) negbeta(32) negone(32)]
B1_OFF, W2_OFF, B2_OFF, NB_OFF, NO_OFF = 0, 32, 128, 160, 192
WB_COLS = 224

# scan columns handled by GPSIMD (0 = DVE only); must be a multiple of N_HID.
GP_COLS = int(os.environ.get("KERNEL_GP_COLS", "416"))
# 1 = ACT also does the beta-scale mult for the GPSIMD lane (pool: 2 TT/step)
ACT_MULT = int(os.environ.get("KERNEL_ACT_MULT", "0"))


def build(nc, n_rows_core=BC, num_steps=NUM_STEPS, gp_cols=GP_COLS):
    nch = n_rows_core // P
    free = nch * N_HID
    assert gp_cols % N_HID == 0 and 0 <= gp_cols < free
    dv_cols = free - gp_cols

    xt_d = nc.dram_tensor("xT", [N_IN, n_rows_core], F32, kind="ExternalInput")
    w1t_d = nc.dram_tensor("w1t", [N_IN, N_HID], F32, kind="ExternalInput")
    wb_d = nc.dram_tensor("wb", [P, WB_COLS], F32, kind="ExternalInput")
    y_d = nc.dram_tensor("y", [n_rows_core, N_OUT], F32, kind="ExternalOutput")

    y_view = y_d[:].rearrange("(p i) o -> p (i o)", p=P)

    dve, gps = nc.vector, nc.gpsimd

    with tile.TileContext(nc) as tc:
        with tc.tile_pool(name="pool", bufs=1) as pool, \
             tc.tile_pool(name="ps", bufs=1, space="PSUM") as psp:
            xt = pool.tile([N_IN, n_rows_core], F32, tag="xt")
            nc.sync.dma_start(xt[:], xt_d[:])
            w1t = pool.tile([N_IN, N_HID], F32, tag="w1t")
            nc.sync.dma_start(w1t[:], w1t_d[:])
            wt = pool.tile([P, WB_COLS], F32, tag="wt")
            nc.sync.dma_start(wt[:], wb_d[:])

            # per-lane state tiles: (cn, n, A[, h]) per engine lane
            lanes = []  # (eng, col0, ncols, cn, nt, at, ht)
            cn_d = pool.tile([P, dv_cols], F32, tag="cn_d")
            nt_d = pool.tile([P, dv_cols], F32, tag="nt_d")
            at_d = pool.tile([P, dv_cols], F32, tag="at_d")
            lanes.append((dve, 0, dv_cols, cn_d, nt_d, at_d, None))
            if gp_cols:
                cn_g = pool.tile([P, gp_cols], F32, tag="cn_g")
                nt_g = pool.tile([P, gp_cols], F32, tag="nt_g")
                at_g = pool.tile([P, gp_cols], F32, tag="at_g")
                ht_g = pool.tile([P, gp_cols], F32, tag="ht_g")
                lanes.append((gps, dv_cols, gp_cols, cn_g, nt_g, at_g, ht_g))

            ot = pool.tile([P, nch * N_OUT], F32, tag="ot")

            b1t = wt[:, B1_OFF : B1_OFF + 32]
            negone = wt[:, NO_OFF : NO_OFF + 1]

            def cbc(off, blocks):
                # [P, 32] const slice -> [P, blocks, 32] broadcast view
                return (
                    wt[:, off : off + 32].unsqueeze(1).broadcast_to([P, blocks, N_HID])
                )

            def h3(ap, cs):
                return ap.rearrange("p (i h) -> p i h", h=N_HID)

            # --- cur1 via PE (bit-exact vs reference), negated+biased into cn ---
            ps = psp.tile([P, free], F32, tag="psA")
            for ch in range(nch):
                nc.tensor.matmul(
                    ps[:, ch * N_HID : (ch + 1) * N_HID],
                    xt[:, ch * P : (ch + 1) * P], w1t[:],
                    start=True, stop=True,
                )
            # cn = (mm * -1) - b1 = -(mm + b1) = -cur1   (DVE reads PSUM)
            for eng, c0, cs, cn, nt, at, ht in lanes:
                ib = cs // N_HID
                dve.scalar_tensor_tensor(
                    h3(cn[:], cs), h3(ps[:, c0 : c0 + cs], cs), -1.0,
                    cbc(B1_OFF, ib), ALU.mult, ALU.subtract,
                )
                # n = cn (membrane after step 1, negated)
                nc.scalar.copy(nt[:], cn[:])

            # --- scan steps 2..num_steps ---
            for _ in range(num_steps - 1):
                for eng, c0, cs, cn, nt, at, ht in lanes:
                    ib = cs // N_HID
                    if eng is dve:
                        eng.scalar_tensor_tensor(
                            at[:], nt[:], -BETA, cn[:], ALU.mult, ALU.subtract
                        )
                        eng.scalar_tensor_tensor(
                            nt[:], nt[:], -THR, at[:], ALU.is_lt, ALU.subtract
                        )
                    else:
                        # h = [n < -1] = Relu(Sign(-n - 1)) on the (idle) ACT
                        # engine: Sign/Relu are exact (Sign(0)=0 verified on
                        # HW), so h is bit-exact. Pool does the arithmetic.
                        nc.scalar.activation(
                            ht[:], nt[:], AF.Sign, bias=negone, scale=-1.0
                        )
                        nc.scalar.activation(ht[:], ht[:], AF.Relu)
                        if ACT_MULT:
                            nc.scalar.mul(at[:], nt[:], -BETA)
                        else:
                            eng.tensor_tensor(
                                h3(at[:], cs), h3(nt[:], cs), cbc(NB_OFF, ib),
                                ALU.mult,
                            )
                        eng.tensor_tensor(at[:], at[:], cn[:], ALU.subtract)
                        eng.tensor_tensor(nt[:], ht[:], at[:], ALU.subtract)

            # --- spike + fc2 ---
            ov = ot[:].rearrange("p (i o) -> p o i", o=N_OUT)
            for eng, c0, cs, cn, nt, at, ht in lanes:
                ib = cs // N_HID
                if eng is dve:
                    eng.tensor_scalar(at[:], nt[:], -THR, None, ALU.is_lt)
                else:
                    nc.scalar.activation(
                        at[:], nt[:], AF.Sign, bias=negone, scale=-1.0
                    )
                    nc.scalar.activation(at[:], at[:], AF.Relu)
            for o in range(N_OUT):
                for eng, c0, cs, cn, nt, at, ht in lanes:
                    ib = cs // N_HID
                    i0 = c0 // N_HID
                    eng.tensor_tensor(
                        h3(cn[:], cs), h3(at[:], cs), cbc(W2_OFF + 32 * o, ib),
                        ALU.mult,
                    )
                    dve.tensor_reduce(
                        ov[:, o : o + 1, i0 : i0 + ib], h3(cn[:], cs),
                        mybir.AxisListType.X, ALU.add,
                    )
                dve.tensor_scalar(
                    ov[:, o : o + 1, :], ov[:, o : o + 1, :],
                    wt[:, B2_OFF + o : B2_OFF + o + 1], None, ALU.add,
                )

            nc.sync.dma_start(y_view, ot[:])
    return nc


_CACHE = {}


def _get_program():
    if "nc" not in _CACHE:
        nc = bacc.Bacc("TRN2", target_bir_lowering=False, debug=False,
                       num_devices=N_CORES)
        build(nc)
        nc.compile()
        _CACHE["nc"] = nc
    return _CACHE["nc"]


def make_wb(b1, W2, b2):
    wb = np.zeros((P, WB_COLS), dtype=np.float32)
    wb[:, B1_OFF : B1_OFF + 32] = b1
    wb[:, W2_OFF : W2_OFF + 96] = np.ascontiguousarray(W2).reshape(-1)
    wb[:, B2_OFF : B2_OFF + 3] = b2
    wb[:, NB_OFF : NB_OFF + 32] = np.float32(-BETA)
    wb[:, NO_OFF : NO_OFF + 32] = np.float32(-THR)
    return wb


def kernel(x, W1, b1, W2, b2):
    x = np.asarray(x, dtype=np.float32)
    W1, b1, W2, b2 = (np.asarray(a, dtype=np.float32) for a in (W1, b1, W2, b2))
    wb = make_wb(b1, W2, b2)
    w1t = np.ascontiguousarray(W1.T)
    nc = _get_program()
    in_maps = [
        {
            "xT": np.ascontiguousarray(x[i * BC : (i + 1) * BC].T),
            "w1t": w1t,
            "wb": wb,
        }
        for i in range(N_CORES)
    ]
    kwargs = dict(_CACHE.get("run_kwargs") or {})
    res = run_bass_kernel_spmd(nc, in_maps, core_ids=list(range(N_CORES)), **kwargs)
    _CACHE["last_results"] = res
    # y rows are stored permuted: dram row p*NCH + ch  <->  logical row ch*P + p
    out = np.empty((B, N_OUT), dtype=np.float32)
    for i in range(N_CORES):
        yc = res.results[i]["y"].reshape(P, NCH, N_OUT)
        out[i * BC : (i + 1) * BC] = yc.transpose(1, 0, 2).reshape(BC, N_OUT)
    return out



# revision 2
# speedup vs baseline: 1.4543x; 1.4543x over previous
"""Trainium2 Bass kernel for the SNN Leaky-Integrate-Fire problem.

Four-engine balanced scan. Per core: rows at [p, chunk] layout, state
n = -mem, cn = -cur1, free dim = 64 chunks x 32 hidden = 2048 cols.

Lanes (columns split by chunks, each lane owns disjoint tiles so the
engines never false-synchronize):
  A (DVE only):        at = (n*-beta) - cn ; n' = (n is_lt -1) - at
  B (PE + DVE):        PSUM = beta*I@n + I@cn (2 fp32 matmuls, 2 bank
                       halves B0/B1); n' = (n is_lt -1) + PSUM
  F (DVE+ACT+Pool):    an = (n*beta) + cn [DVE]; h = Relu(Sign(-n-1))
                       [ACT]; n' = h + an [Pool]. Split into halves
                       F0/F1 so the Pool->ACT->Pool chain pipelines.

cn is computed elementwise on DVE (6 TT ops with broadcast views of
-W1 rows / -b1), nt inits are ACT copies. Numerics: DVE/ACT/Pool paths
reproduce the reference's fl(fl(beta*m)+c)-h roundings exactly; the PE
lane's fl(beta*n) occasionally differs by 1ulp (double rounding), which
costs O(10) wrong spikes out of 2M -- far inside the 2e-2 gate.

fc2: spikes per lane; per-o TT mult with broadcast W2 row + DVE
tensor_reduce into ov; bias add; DMA out. Host inverse-permutes rows.
"""
import os
import sys

sys.path.insert(0, "/opt/trn_rl_repo")

import numpy as np

import concourse.bacc as bacc
import concourse.tile as tile
from concourse import mybir
from concourse.bass_utils import run_bass_kernel_spmd

F32 = mybir.dt.float32
ALU = mybir.AluOpType
AF = mybir.ActivationFunctionType

# problem constants (hardcoded per contract)
B, N_IN, N_HID, N_OUT = 65536, 3, 32, 3
NUM_STEPS, BETA, THR = 100, 0.9, 1.0
N_CORES = 8
BC = B // N_CORES          # rows per core = 8192
P = 128                    # partitions
NCH = BC // P              # 128-row chunks per core = 64

# lane splits in chunks (32 cols each): A | F0 F1 | B0 B1
CA = int(os.environ.get("KERNEL_CA", "8"))
CB = int(os.environ.get("KERNEL_CB", "21"))
CF = NCH - CA - CB
CF0 = CF // 2
CF1 = CF - CF0
CB0 = (CB + 1) // 2
CB1 = CB - CB0

# wb const layout [P, WB_COLS] (values replicated across partitions):
# [w1n(3*32) b1n(32) w2(3*32) b2(3) negone(1) pad]
W1N_OFF = 0
B1N_OFF = 96
W2_OFF = 128
B2_OFF = 224
NO_OFF = 227
WB_COLS = 256


def build(nc, num_steps=NUM_STEPS):
    xA, xB, xF = CA * N_HID, CB * N_HID, CF * N_HID
    xB0, xB1 = CB0 * N_HID, CB1 * N_HID
    xF0, xF1 = CF0 * N_HID, CF1 * N_HID

    xaf_d = nc.dram_tensor("xk", [P, N_IN * NCH], F32, kind="ExternalInput")
    wb_d = nc.dram_tensor("wb", [P, WB_COLS], F32, kind="ExternalInput")
    eye_d = nc.dram_tensor("eye", [P, 2 * P], F32, kind="ExternalInput")
    y_d = nc.dram_tensor("y", [BC, N_OUT], F32, kind="ExternalOutput")
    y_view = y_d[:].rearrange("(p i) o -> p (i o)", p=P)

    dve, gps, act = nc.vector, nc.gpsimd, nc.scalar

    # chunk ranges: A: [0, CA), F0: [CA, CA+CF0), F1: [..+CF1), B0, B1
    c_a = 0
    c_f0 = CA
    c_f1 = CA + CF0
    c_b0 = CA + CF
    c_b1 = CA + CF + CB0

    def h3(ap, cols):
        return ap.rearrange("p (i h) -> p i h", h=N_HID)

    with tile.TileContext(nc) as tc:
        with tc.tile_pool(name="pool", bufs=1) as pool, \
             tc.tile_pool(name="ps", bufs=1, space="PSUM") as psp:
            xk = pool.tile([P, N_IN * NCH], F32, tag="xk")
            nc.sync.dma_start(xk[:], xaf_d[:])
            wt = pool.tile([P, WB_COLS], F32, tag="wt")
            nc.sync.dma_start(wt[:], wb_d[:])
            eye = pool.tile([P, 2 * P], F32, tag="eye")
            nc.sync.dma_start(eye[:], eye_d[:])

            cn = pool.tile([P, NCH * N_HID], F32, tag="cn")
            sc = pool.tile([P, NCH * N_HID], F32, tag="sc")  # cn scratch

            def wbc(off, blocks):
                return (
                    wt[:, off: off + N_HID].unsqueeze(1)
                    .broadcast_to([P, blocks, N_HID])
                )

            def xbv(k):
                return (
                    xk[:, k * NCH: (k + 1) * NCH].unsqueeze(2)
                    .broadcast_to([P, NCH, N_HID])
                )

            # ---- cn = -cur1 = sum_k x_k*(-W1[:,k]) + (-b1), 6 DVE TT ops
            cn3 = h3(cn[:], NCH * N_HID)
            sc3 = h3(sc[:], NCH * N_HID)
            dve.tensor_tensor(cn3, xbv(0), wbc(W1N_OFF, NCH), ALU.mult)
            dve.tensor_tensor(sc3, xbv(1), wbc(W1N_OFF + N_HID, NCH), ALU.mult)
            dve.tensor_tensor(cn3, cn3, sc3, ALU.add)
            dve.tensor_tensor(sc3, xbv(2), wbc(W1N_OFF + 2 * N_HID, NCH), ALU.mult)
            dve.tensor_tensor(cn3, cn3, sc3, ALU.add)
            dve.tensor_tensor(cn3, cn3, wbc(B1N_OFF, NCH), ALU.add)

            # ---- per-lane state tiles + nt init (ACT copies of cn slices)
            nt_a = pool.tile([P, xA], F32, tag="nt_a")
            at_a = pool.tile([P, xA], F32, tag="at_a")
            nt_b = pool.tile([P, xB], F32, tag="nt_b")
            sp_b = pool.tile([P, xB], F32, tag="sp_b")
            nt_f0 = pool.tile([P, xF0], F32, tag="nt_f0")
            an_f0 = pool.tile([P, xF0], F32, tag="an_f0")
            ht_f0 = pool.tile([P, xF0], F32, tag="ht_f0")
            sc_f0 = pool.tile([P, xF0], F32, tag="sc_f0")
            nt_f1 = pool.tile([P, xF1], F32, tag="nt_f1")
            an_f1 = pool.tile([P, xF1], F32, tag="an_f1")
            ht_f1 = pool.tile([P, xF1], F32, tag="ht_f1")
            sc_f1 = pool.tile([P, xF1], F32, tag="sc_f1")
            ov = pool.tile([P, NCH * N_OUT], F32, tag="ov")

            ps0 = psp.tile([P, xB0], F32, tag="ps0")
            ps1 = psp.tile([P, xB1], F32, tag="ps1")

            cA0 = c_a * N_HID
            cF0o = c_f0 * N_HID
            cF1o = c_f1 * N_HID
            cB0o = c_b0 * N_HID
            cB1o = c_b1 * N_HID

            act.copy(nt_a[:], cn[:, cA0: cA0 + xA])
            act.copy(nt_f0[:], cn[:, cF0o: cF0o + xF0])
            act.copy(nt_f1[:], cn[:, cF1o: cF1o + xF1])
            act.copy(nt_b[:], cn[:, cB0o: cB0o + xB])

            cn_a = cn[:, cA0: cA0 + xA]
            cn_f0 = cn[:, cF0o: cF0o + xF0]
            cn_f1 = cn[:, cF1o: cF1o + xF1]
            cn_b0 = cn[:, cB0o: cB0o + xB0]
            cn_b1 = cn[:, cB1o: cB1o + xB1]

            negone = wt[:, NO_OFF: NO_OFF + 1]
            eyeB = eye[:, 0:P]
            eyeI = eye[:, P: 2 * P]

            # ---- scan steps 2..num_steps
            for _ in range(num_steps - 1):
                # PE lane B: PSUM = beta*n + cn
                nc.tensor.matmul(ps0[:], eyeB, nt_b[:, 0:xB0],
                                 start=True, stop=False)
                nc.tensor.matmul(ps0[:], eyeI, cn_b0, start=False, stop=True)
                nc.tensor.matmul(ps1[:], eyeB, nt_b[:, xB0:xB],
                                 start=True, stop=False)
                nc.tensor.matmul(ps1[:], eyeI, cn_b1, start=False, stop=True)
                # DVE lane A
                dve.scalar_tensor_tensor(
                    at_a[:], nt_a[:], -BETA, cn_a, ALU.mult, ALU.subtract)
                dve.scalar_tensor_tensor(
                    nt_a[:], nt_a[:], -THR, at_a[:], ALU.is_lt, ALU.subtract)
                # DVE lane B consume
                dve.scalar_tensor_tensor(
                    nt_b[:, 0:xB0], nt_b[:, 0:xB0], -THR, ps0[:],
                    ALU.is_lt, ALU.add)
                dve.scalar_tensor_tensor(
                    nt_b[:, xB0:xB], nt_b[:, xB0:xB], -THR, ps1[:],
                    ALU.is_lt, ALU.add)
                # lane F halves
                dve.scalar_tensor_tensor(
                    an_f0[:], nt_f0[:], BETA, cn_f0, ALU.mult, ALU.add)
                act.activation(ht_f0[:], nt_f0[:], AF.Sign,
                               bias=negone, scale=-1.0)
                act.activation(ht_f0[:], ht_f0[:], AF.Relu)
                gps.tensor_tensor(nt_f0[:], ht_f0[:], an_f0[:], ALU.add)
                dve.scalar_tensor_tensor(
                    an_f1[:], nt_f1[:], BETA, cn_f1, ALU.mult, ALU.add)
                act.activation(ht_f1[:], nt_f1[:], AF.Sign,
                               bias=negone, scale=-1.0)
                act.activation(ht_f1[:], ht_f1[:], AF.Relu)
                gps.tensor_tensor(nt_f1[:], ht_f1[:], an_f1[:], ALU.add)

            # ---- spikes
            dve.tensor_scalar(at_a[:], nt_a[:], -THR, None, ALU.is_lt)
            dve.tensor_scalar(sp_b[:], nt_b[:], -THR, None, ALU.is_lt)
            act.activation(ht_f0[:], nt_f0[:], AF.Sign, bias=negone, scale=-1.0)
            act.activation(ht_f0[:], ht_f0[:], AF.Relu)
            act.activation(ht_f1[:], nt_f1[:], AF.Sign, bias=negone, scale=-1.0)
            act.activation(ht_f1[:], ht_f1[:], AF.Relu)

            # ---- fc2: out[p, i, o] = sum_h spk * W2[o, h] (+ b2)
            ovv = ov[:].rearrange("p (i o) -> p o i", o=N_OUT)
            # Pool products for F (3 per half, distinct dead tiles)
            fprod = [(an_f0, nt_f0, sc_f0), (an_f1, nt_f1, sc_f1)]
            for hh, (half, c0, xFh) in enumerate(
                    [(ht_f0, c_f0, xF0), (ht_f1, c_f1, xF1)]):
                for o in range(N_OUT):
                    dst = fprod[hh][o]
                    gps.tensor_tensor(
                        h3(dst[:], xFh), h3(half[:], xFh),
                        wbc(W2_OFF + N_HID * o, xFh // N_HID), ALU.mult)
            # DVE: A and B products + reduces
            for o in range(N_OUT):
                dve.tensor_tensor(
                    h3(nt_a[:], xA), h3(at_a[:], xA), wbc(W2_OFF + N_HID * o, CA),
                    ALU.mult)
                dve.tensor_reduce(
                    ovv[:, o: o + 1, c_a: c_a + CA], h3(nt_a[:], xA),
                    mybir.AxisListType.X, ALU.add)
                dve.tensor_tensor(
                    h3(nt_b[:], xB), h3(sp_b[:], xB), wbc(W2_OFF + N_HID * o, CB),
                    ALU.mult)
                dve.tensor_reduce(
                    ovv[:, o: o + 1, c_b0: c_b0 + CB], h3(nt_b[:], xB),
                    mybir.AxisListType.X, ALU.add)
            # DVE: F reduces (Pool products), at the tail of DVE's stream
            for hh, (c0, nchh) in enumerate([(c_f0, CF0), (c_f1, CF1)]):
                for o in range(N_OUT):
                    src = fprod[hh][o]
                    dve.tensor_reduce(
                        ovv[:, o: o + 1, c0: c0 + nchh],
                        h3(src[:], nchh * N_HID), mybir.AxisListType.X, ALU.add)
            # bias
            for o in range(N_OUT):
                dve.tensor_scalar(
                    ovv[:, o: o + 1, :], ovv[:, o: o + 1, :],
                    wt[:, B2_OFF + o: B2_OFF + o + 1], None, ALU.add)

            nc.sync.dma_start(y_view, ov[:])
    return nc


_CACHE = {}


def _get_program():
    if "nc" not in _CACHE:
        nc = bacc.Bacc("TRN2", target_bir_lowering=False, debug=False,
                       num_devices=N_CORES)
        build(nc)
        nc.compile()
        _CACHE["nc"] = nc
    return _CACHE["nc"]


def make_wb(b1, W2, b2):
    wb = np.zeros((P, WB_COLS), dtype=np.float32)
    # negated W1 columns are filled by caller (needs W1); here static parts
    wb[:, W2_OFF: W2_OFF + 3 * N_HID] = np.ascontiguousarray(W2).reshape(-1)
    wb[:, B2_OFF: B2_OFF + N_OUT] = b2
    wb[:, B1N_OFF: B1N_OFF + N_HID] = -b1
    wb[:, NO_OFF] = np.float32(-THR)
    return wb


def kernel(x, W1, b1, W2, b2):
    x = np.asarray(x, dtype=np.float32)
    W1, b1, W2, b2 = (np.asarray(a, dtype=np.float32) for a in (W1, b1, W2, b2))
    wb = make_wb(b1, W2, b2)
    for k in range(N_IN):
        wb[:, W1N_OFF + k * N_HID: W1N_OFF + (k + 1) * N_HID] = -W1[:, k]
    eye = np.zeros((P, 2 * P), dtype=np.float32)
    eye[np.arange(P), np.arange(P)] = np.float32(BETA)
    eye[np.arange(P), P + np.arange(P)] = np.float32(1.0)

    nc = _get_program()
    in_maps = []
    for i in range(N_CORES):
        xs = x[i * BC: (i + 1) * BC]              # [8192, 3]
        X3 = np.ascontiguousarray(xs.T).reshape(N_IN, NCH, P)
        xk = np.ascontiguousarray(X3.transpose(2, 0, 1)).reshape(P, N_IN * NCH)
        in_maps.append({"xk": xk, "wb": wb, "eye": eye})
    kwargs = dict(_CACHE.get("run_kwargs") or {})
    res = run_bass_kernel_spmd(nc, in_maps, core_ids=list(range(N_CORES)), **kwargs)
    _CACHE["last_results"] = res
    # y rows are stored permuted: dram row p*NCH + ch <-> logical row ch*P + p
    out = np.empty((B, N_OUT), dtype=np.float32)
    for i in range(N_CORES):
        yc = res.results[i]["y"].reshape(P, NCH, N_OUT)
        out[i * BC: (i + 1) * BC] = yc.transpose(1, 0, 2).reshape(BC, N_OUT)
    return out
